# revision 1
# baseline (speedup 1.0000x reference)
"""Locally banded sparse attention (window=64) on 8 Trainium2 NeuronCores.

Sequence-parallel: each core owns 256 contiguous query positions and
receives a 384-row x chunk (its 256 rows + 64-row halo on each side,
zero-padded at the sequence edges) plus a full replica of the four
projection matrices.  No device collectives are needed.

Per-core device kernel (all fp32):
  1. qT/kT (head-transposed, d on partitions) and v (natural, keys on
     partitions) projections via PE matmuls.
  2. For each of 2 query tiles x 8 heads: a dense 128x256 score block
     (the 129-wide band of a 128-query tile spans exactly 256 contiguous
     keys), additive band/validity mask, softmax along the free axis,
     PE transpose of the probabilities, and P@V accumulation.
  3. Output projection producing outT [512, 256]; the host transposes
     and concatenates the 8 chunks.
"""

import numpy as np

import concourse.bass as bass
import concourse.tile as tile
from concourse import bacc, mybir
from concourse import bass_utils
from concourse.bass import ts, ds
from concourse.masks import make_identity

F32 = mybir.dt.float32
N_CORES = 8
S = 2048
D = 512
H = 8
DK = 64
W = 64
SCALE = 1.0 / np.sqrt(DK)
SEQ_PER_CORE = S // N_CORES          # 256
CHUNK = SEQ_PER_CORE + 2 * W         # 384 rows of k/v context per core
NEG = -1.0e30

_CACHE = {}


def _build_program():
    nc = bacc.Bacc("TRN2", target_bir_lowering=False, debug=False,
                   num_devices=N_CORES)

    xT = nc.dram_tensor("xT", [D, CHUNK], F32, kind="ExternalInput").ap()
    wqT = nc.dram_tensor("wqT", [D, D], F32, kind="ExternalInput").ap()
    wkT = nc.dram_tensor("wkT", [D, D], F32, kind="ExternalInput").ap()
    wvT = nc.dram_tensor("wvT", [D, D], F32, kind="ExternalInput").ap()
    woT = nc.dram_tensor("woT", [D, D], F32, kind="ExternalInput").ap()
    mask = nc.dram_tensor("mask", [2, 128, 256], F32, kind="ExternalInput").ap()
    # biases, pre-reshaped to [128, 4] (bias[g*128+p] -> [p, g]); bq pre-scaled
    bq = nc.dram_tensor("bq", [128, 4], F32, kind="ExternalInput").ap()
    bk = nc.dram_tensor("bk", [128, 4], F32, kind="ExternalInput").ap()
    bv = nc.dram_tensor("bv", [128, 4], F32, kind="ExternalInput").ap()
    bo = nc.dram_tensor("bo", [128, 4], F32, kind="ExternalInput").ap()
    outT = nc.dram_tensor("outT", [D, SEQ_PER_CORE], F32,
                          kind="ExternalOutput").ap()

    QLO, QHI = W, W + SEQ_PER_CORE   # query rows inside the chunk

    def r32(ap):
        return ap.bitcast(mybir.dt.float32r)

    with tile.TileContext(nc) as tc:
        with (
            tc.tile_pool(name="const", bufs=1) as cpool,
            tc.tile_pool(name="proj_ps", bufs=2, space="PSUM") as proj_ps,
            tc.tile_pool(name="s_ps", bufs=2, space="PSUM") as s_ps,
            tc.tile_pool(name="pt_ps", bufs=2, space="PSUM") as pt_ps,
            tc.tile_pool(name="av_ps", bufs=2, space="PSUM") as av_ps,
            tc.tile_pool(name="soft", bufs=3) as soft,
            tc.tile_pool(name="small", bufs=4) as small,
        ):
            # ---- persistent SBUF tiles -------------------------------
            def persist(shape, tag):
                return cpool.tile(shape, F32, tag=tag, name=tag)

            x_sb = [persist([128, CHUNK], f"x{k}") for k in range(4)]
            wq_sb = [persist([128, D], f"wq{k}") for k in range(4)]
            wk_sb = [persist([128, D], f"wk{k}") for k in range(4)]
            wv_sb = [persist([128, D], f"wv{k}") for k in range(4)]
            wo_sb = [persist([128, D], f"wo{k}") for k in range(4)]
            m_sb = [persist([128, 256], f"m{t}") for t in range(2)]
            q_sb = [persist([128, SEQ_PER_CORE], f"q{g}") for g in range(4)]
            k_sb = [persist([128, CHUNK], f"k{g}") for g in range(4)]
            v_sb = [persist([128, D], f"v{r}") for r in range(3)]
            a_sb = [persist([128, SEQ_PER_CORE], f"a{g}") for g in range(4)]
            o_sb = [persist([128, SEQ_PER_CORE], f"o{g}") for g in range(4)]
            bq_sb = persist([128, 4], "bq")
            bk_sb = persist([128, 4], "bk")
            bv_sb = persist([128, 4], "bv")
            bo_sb = persist([128, 4], "bo")
            ident = persist([128, 128], "ident")
            identr = persist([128, 128], "identr")

            make_identity(nc, ident[:])
            nc.vector.tensor_copy(r32(identr[:]), ident[:])

            # input DMAs in consumption order
            for k in range(4):
                nc.sync.dma_start(r32(x_sb[k][:]), r32(xT[ts(k, 128), :]))
            for k in range(4):
                nc.sync.dma_start(r32(wk_sb[k][:]), r32(wkT[ts(k, 128), :]))
            nc.sync.dma_start(bk_sb[:], bk[:, :])
            for k in range(4):
                nc.sync.dma_start(r32(wq_sb[k][:]), r32(wqT[ts(k, 128), :]))
            nc.sync.dma_start(bq_sb[:], bq[:, :])
            for k in range(4):
                nc.sync.dma_start(r32(wv_sb[k][:]), r32(wvT[ts(k, 128), :]))
            for t in range(2):
                nc.sync.dma_start(m_sb[t][:], mask[t, :, :])
            nc.sync.dma_start(bv_sb[:], bv[:, :])
            for k in range(4):
                nc.sync.dma_start(r32(wo_sb[k][:]), r32(woT[ts(k, 128), :]))
            nc.sync.dma_start(bo_sb[:], bo[:, :])

            # ---- projections ----------------------------------------
            # kT[o, j] / qT[o, r]: accumulate over d-chunks kk
            for g in range(4):
                ps = proj_ps.tile([128, 512], F32, tag="proj", name="proj")
                for kk in range(4):
                    nc.tensor.matmul(ps[:, :CHUNK], r32(wk_sb[kk][:, ts(g, 128)]),
                                     r32(x_sb[kk][:]), start=(kk == 0),
                                     stop=(kk == 3))
                nc.scalar.activation(r32(k_sb[g][:]), ps[:, :CHUNK],
                                     mybir.ActivationFunctionType.Identity,
                                     bias=bk_sb[:, g:g + 1])
            for g in range(4):
                ps = proj_ps.tile([128, 512], F32, tag="proj", name="proj")
                for kk in range(4):
                    nc.tensor.matmul(ps[:, :SEQ_PER_CORE],
                                     r32(wq_sb[kk][:, ts(g, 128)]),
                                     r32(x_sb[kk][:, QLO:QHI]), start=(kk == 0),
                                     stop=(kk == 3))
                # q is pre-scaled by 1/sqrt(dk); bq arrives pre-scaled too
                nc.scalar.activation(r32(q_sb[g][:]), ps[:, :SEQ_PER_CORE],
                                     mybir.ActivationFunctionType.Identity,
                                     bias=bq_sb[:, g:g + 1], scale=SCALE)
            # v natural ([keys, d]): lhsT = x chunk cols, rhs = wvT
            for r in range(3):
                ps = proj_ps.tile([128, 512], F32, tag="proj", name="proj")
                for kk in range(4):
                    nc.tensor.matmul(ps[:], r32(x_sb[kk][:, ts(r, 128)]),
                                     r32(wv_sb[kk][:]),
                                     start=(kk == 0), stop=(kk == 3))
                nc.scalar.activation(r32(v_sb[r][:]), ps[:],
                                     mybir.ActivationFunctionType.Identity)

            # ---- banded attention -----------------------------------
            for t in range(2):
                for h in range(8):
                    g, po = h // 2, (h % 2) * 64
                    sps = s_ps.tile([128, 256], F32, tag="s", name="s")
                    nc.tensor.matmul(sps[:],
                                     r32(q_sb[g][ds(po, 64), ts(t, 128)]),
                                     r32(k_sb[g][ds(po, 64), ds(t * 128, 256)]),
                                     start=True, stop=True)
                    p = soft.tile([128, 256], F32, tag="p", name="p")
                    nc.vector.tensor_add(p[:], sps[:], m_sb[t][:])
                    # max-free softmax: scores are O(10), exp safe in fp32
                    rsum = small.tile([128, 1], F32, tag="rsum", name="rsum")
                    nc.scalar.activation(p[:], p[:],
                                         mybir.ActivationFunctionType.Exp,
                                         accum_out=rsum[:])
                    rcp = small.tile([128, 1], F32, tag="rcp", name="rcp")
                    nc.vector.reciprocal(rcp[:], rsum[:])
                    pn = soft.tile([128, 256], F32, tag="pn", name="pn")
                    nc.vector.tensor_scalar_mul(r32(pn[:]), p[:], rcp[:])
                    ptp = pt_ps.tile([128, 256], F32, tag="pt", name="pt")
                    for jb in range(2):
                        nc.tensor.transpose(r32(ptp[:, ts(jb, 128)]),
                                            r32(pn[:, ts(jb, 128)]),
                                            r32(identr[:]))
                    pts = soft.tile([128, 256], F32, tag="pts", name="pts")
                    nc.vector.tensor_copy(r32(pts[:]), ptp[:])
                    av = av_ps.tile([64, 128], F32, tag="av", name="av")
                    for jb in range(2):
                        nc.tensor.matmul(av[:],
                                         r32(v_sb[t + jb][:, ds(h * DK, DK)]),
                                         r32(pts[:, ts(jb, 128)]),
                                         start=(jb == 0), stop=(jb == 1))
                    nc.scalar.activation(r32(a_sb[g][ds(po, 64), ts(t, 128)]),
                                         av[:],
                                         mybir.ActivationFunctionType.Identity,
                                         bias=bv_sb[ds(po, 64), g:g + 1])

            # ---- output projection ----------------------------------
            for g in range(4):
                ps = proj_ps.tile([128, 512], F32, tag="proj", name="proj")
                for kk in range(4):
                    nc.tensor.matmul(ps[:, :SEQ_PER_CORE],
                                     r32(wo_sb[kk][:, ts(g, 128)]),
                                     r32(a_sb[kk][:]), start=(kk == 0),
                                     stop=(kk == 3))
                nc.scalar.activation(o_sb[g][:], ps[:, :SEQ_PER_CORE],
                                     mybir.ActivationFunctionType.Identity,
                                     bias=bo_sb[:, g:g + 1])
                nc.sync.dma_start(outT[ts(g, 128), :], o_sb[g][:])

    nc.compile()
    return nc


def _band_mask(tile_idx):
    """Additive mask [128, 256] for global query tile `tile_idx` (0..15)."""
    r = np.arange(128)[:, None]
    j = np.arange(256)[None, :]
    ok = (j >= r) & (j <= r + 2 * W)
    key_global = tile_idx * 128 - W + j
    ok &= (key_global >= 0) & (key_global < S)
    return np.where(ok, 0.0, NEG).astype(np.float32)


def _prep_inputs(x, Wq, bq, Wk, bk, Wv, bv, Wo, bo):
    wqT = np.ascontiguousarray(Wq.T.astype(np.float32))
    wkT = np.ascontiguousarray(Wk.T.astype(np.float32))
    wvT = np.ascontiguousarray(Wv.T.astype(np.float32))
    woT = np.ascontiguousarray(Wo.T.astype(np.float32))

    def resh(b, scale=1.0):
        return np.ascontiguousarray(
            (np.asarray(b, np.float32) * scale).reshape(4, 128).T)

    bq_r, bk_r = resh(bq, SCALE), resh(bk)
    bv_r, bo_r = resh(bv), resh(bo)

    in_maps = []
    for c in range(N_CORES):
        klo = c * SEQ_PER_CORE - W
        lo, hi = max(0, klo), min(S, klo + CHUNK)
        xT_c = np.zeros((D, CHUNK), np.float32)
        xT_c[:, lo - klo:hi - klo] = x[0, lo:hi, :].T
        m = np.stack([_band_mask(c * 2), _band_mask(c * 2 + 1)])
        in_maps.append({
            "xT": np.ascontiguousarray(xT_c), "mask": np.ascontiguousarray(m),
            "wqT": wqT, "wkT": wkT, "wvT": wvT, "woT": woT,
            "bq": bq_r, "bk": bk_r, "bv": bv_r, "bo": bo_r,
        })
    return in_maps


def kernel(x, Wq, bq, Wk, bk, Wv, bv, Wo, bo):
    if "nc" not in _CACHE:
        _CACHE["nc"] = _build_program()
    nc = _CACHE["nc"]
    in_maps = _prep_inputs(x, Wq, bq, Wk, bk, Wv, bv, Wo, bo)
    res = bass_utils.run_bass_kernel_spmd(nc, in_maps,
                                          core_ids=list(range(N_CORES)))
    out = np.empty((1, S, D), np.float32)
    for c in range(N_CORES):
        out[0, c * SEQ_PER_CORE:(c + 1) * SEQ_PER_CORE, :] = \
            res.results[c]["outT"].T
    return out



# revision 13
# speedup vs baseline: 1.0077x; 1.0077x over previous
"""Locally banded sparse attention (window=64) on 8 Trainium2 NeuronCores.

Sequence-parallel: each core owns 256 contiguous query positions and
receives a 384-row x chunk (its 256 rows + 64-row halo on each side,
zero-padded at the sequence edges) plus a full replica of the four
projection matrices.  No device collectives are needed.

All matmuls run in bf16 (fp32 PSUM accumulation).  Attention scores are
computed directly in transposed layout S^T[key, query] = kT.T @ qT so the
probabilities never need a PE transpose before the P@V matmul.  Softmax
denominators ride along as a ones-column appended to V (row 64 of the AV
output); normalization is a K=2 selector matmul that broadcasts the two
heads' reciprocal sums across partitions, followed by one multiply.

Host-side folds: SCALE and bq into Wq/bq, bv into an effective bo
(out += bv @ Wo.T is query-independent).
"""

import numpy as np
import ml_dtypes

import concourse.bass as bass
import concourse.tile as tile
from concourse import bacc, mybir
from concourse import bass_utils

F32 = mybir.dt.float32
BF16 = mybir.dt.bfloat16
N_CORES = 8
S = 2048
D = 512
H = 8
DK = 64
W = 64
SCALE = 1.0 / np.sqrt(DK)
SEQ_PER_CORE = S // N_CORES          # 256
CHUNK = SEQ_PER_CORE + 2 * W         # 384 rows of k/v context per core

_CACHE = {}


def _build_program():
    nc = bacc.Bacc("TRN2", target_bir_lowering=False, debug=False,
                   num_devices=N_CORES)

    # packed DRAM inputs (bf16): weight chunk kk lives at cols [kk*512, +512)
    x4 = nc.dram_tensor("x4", [128, 4 * CHUNK], BF16, kind="ExternalInput").ap()
    wk4 = nc.dram_tensor("wk4", [128, 2048], BF16, kind="ExternalInput").ap()
    wq4 = nc.dram_tensor("wq4", [128, 2048], BF16, kind="ExternalInput").ap()
    wv4 = nc.dram_tensor("wv4", [128, 2048], BF16, kind="ExternalInput").ap()
    wo4 = nc.dram_tensor("wo4", [128, 2048], BF16, kind="ExternalInput").ap()
    # binary band masks, S^T layout: block (t, kb) at cols [(t*2+kb)*128)
    mask4 = nc.dram_tensor("mask4", [128, 512], BF16, kind="ExternalInput").ap()
    # col 0-3: bk[g]; col 4-7: bo_eff[g]; col 8-11: bq_scaled[g]
    bias = nc.dram_tensor("bias", [128, 12], F32, kind="ExternalInput").ap()
    outT = nc.dram_tensor("outT", [128, 4 * SEQ_PER_CORE], BF16,
                          kind="ExternalOutput").ap()

    with tile.TileContext(nc) as tc:
        with (
            tc.tile_pool(name="const", bufs=1) as cpool,
            tc.tile_pool(name="pp", bufs=2, space="PSUM") as pp,
            tc.tile_pool(name="s_ps", bufs=4, space="PSUM") as s_ps,
            tc.tile_pool(name="av_ps", bufs=2, space="PSUM") as av_ps,
            tc.tile_pool(name="soft", bufs=4) as soft,
            tc.tile_pool(name="small", bufs=4) as small,
        ):
            def persist(shape, tag, dtype=BF16):
                return cpool.tile(shape, dtype, tag=tag, name=tag)

            x_sb = persist([128, 4 * CHUNK], "x")
            wk_sb = persist([128, 2048], "wk")
            wq_sb = persist([128, 2048], "wq")
            wv_sb = persist([128, 2048], "wv")
            wo_sb = persist([128, 2048], "wo")
            mask_sb = persist([128, 512], "mask")
            bias_sb = persist([128, 12], "bias", F32)
            k_sb = [persist([128, CHUNK], f"k{g}") for g in range(4)]
            q_sb = [persist([128, SEQ_PER_CORE], f"q{g}") for g in range(4)]
            # v with a ones column per head: head h at cols [h*65, +64], ones at h*65+64
            vaug = [persist([128, 8 * 65], f"v{r}") for r in range(3)]
            a_sb = [persist([128, SEQ_PER_CORE], f"a{g}") for g in range(4)]
            o_sb = [persist([128, SEQ_PER_CORE], f"o{g}") for g in range(4)]
            onesr = persist([1, 64], "onesr")

            # input DMAs in consumption order
            nc.sync.dma_start(bias_sb[:], bias[:, :])
            nc.sync.dma_start(x_sb[:], x4[:, :])
            nc.sync.dma_start(wk_sb[:], wk4[:, :])
            nc.sync.dma_start(wq_sb[:], wq4[:, :])
            nc.sync.dma_start(wv_sb[:], wv4[:, :])
            nc.sync.dma_start(mask_sb[:], mask4[:, :])
            nc.sync.dma_start(wo_sb[:], wo4[:, :])

            # ones row for broadcasting each head's 1/Z across 64 partitions
            nc.vector.memset(onesr[:], 1.0)
            # ones columns of vaug (col h*65+64 for each head)
            def vaug_ap(r, col0, ncols):
                base = vaug[r][:]
                p_step = base.ap[0][0]
                return bass.AP(base.tensor, base.offset + col0,
                               [[p_step, 128], [65, 8], [1, ncols]])

            for r in range(3):
                nc.vector.memset(vaug_ap(r, 64, 1), 1.0)

            # ---- projections ----------------------------------------
            # kT[g]: [128 dout, 384 keys], bf16, + bk
            for g in range(4):
                ps = pp.tile([128, 512], F32, tag="pp", name="pp")
                for kk in range(4):
                    nc.tensor.matmul(ps[:, :CHUNK],
                                     wk_sb[:, kk * 512 + g * 128:
                                           kk * 512 + g * 128 + 128],
                                     x_sb[:, kk * CHUNK:(kk + 1) * CHUNK],
                                     start=(kk == 0), stop=(kk == 3))
                nc.vector.tensor_scalar_add(k_sb[g][:], ps[:, :CHUNK],
                                            bias_sb[:, g:g + 1])
            # qT[g]: [128 dout, 256 queries] (SCALE and bq pre-folded on host)
            for g in range(4):
                ps = pp.tile([128, 512], F32, tag="pp", name="pp")
                for kk in range(4):
                    nc.tensor.matmul(ps[:, :SEQ_PER_CORE],
                                     wq_sb[:, kk * 512 + g * 128:
                                           kk * 512 + g * 128 + 128],
                                     x_sb[:, kk * CHUNK + W:
                                          kk * CHUNK + W + SEQ_PER_CORE],
                                     start=(kk == 0), stop=(kk == 3))
                nc.vector.tensor_scalar_add(q_sb[g][:], ps[:, :SEQ_PER_CORE],
                                            bias_sb[:, 8 + g:9 + g])
            # v natural [keys, dout], strided-copied into vaug (65-wide heads)
            for r in range(3):
                ps = pp.tile([128, 512], F32, tag="pp", name="pp")
                for kk in range(4):
                    nc.tensor.matmul(ps[:],
                                     x_sb[:, kk * CHUNK + r * 128:
                                          kk * CHUNK + r * 128 + 128],
                                     wv_sb[:, kk * 512:(kk + 1) * 512],
                                     start=(kk == 0), stop=(kk == 3))
                nc.scalar.activation(vaug_ap(r, 0, 64), ps[:],
                                     mybir.ActivationFunctionType.Copy)

            # ---- banded attention (S^T layout) ----------------------
            for g in range(4):
                hA, hB = 2 * g, 2 * g + 1
                for t in range(2):
                    # separate banks: concurrent row-tiled MMs must not
                    # write the same PSUM bank (hardware hang)
                    sA = s_ps.tile([128, 256], F32, tag="s", name="s")
                    sB = s_ps.tile([128, 256], F32, tag="s", name="s")
                    sA, sB = sA[:, :], sB[:, :]
                    for kb in range(2):
                        kc = (t + kb) * 128
                        nc.tensor.matmul(sA[:, kb * 128:(kb + 1) * 128],
                                         k_sb[g][0:64, kc:kc + 128],
                                         q_sb[g][0:64, t * 128:(t + 1) * 128],
                                         start=True, stop=True)
                        nc.tensor.matmul(sB[:, kb * 128:(kb + 1) * 128],
                                         k_sb[g][64:128, kc:kc + 128],
                                         q_sb[g][64:128, t * 128:(t + 1) * 128],
                                         start=True, stop=True)
                    eA = soft.tile([128, 256], BF16, tag="eA", name="eA")
                    eB = soft.tile([128, 256], BF16, tag="eB", name="eB")
                    nc.scalar.activation(eA[:], sA,
                                         mybir.ActivationFunctionType.Exp)
                    nc.scalar.activation(eB[:], sB,
                                         mybir.ActivationFunctionType.Exp)
                    pA = soft.tile([128, 256], BF16, tag="pA", name="pA")
                    pB = soft.tile([128, 256], BF16, tag="pB", name="pB")
                    msk = mask_sb[:, t * 256:(t + 1) * 256]
                    nc.vector.tensor_mul(pA[:], eA[:], msk)
                    nc.vector.tensor_mul(pB[:], eB[:], msk)
                    # one PSUM bank: psA | psB | bps, groups kept sequential
                    av = av_ps.tile([128, 384], F32, tag="av", name="av")
                    psA, psB = av[0:65, 0:128], av[0:65, 128:256]
                    bps = av[:, 256:384]
                    for kb in range(2):
                        nc.tensor.matmul(psA,
                                         vaug[t + kb][:, hA * 65:hA * 65 + 65],
                                         pA[:, kb * 128:(kb + 1) * 128],
                                         start=(kb == 0), stop=(kb == 1))
                    for kb in range(2):
                        nc.tensor.matmul(psB,
                                         vaug[t + kb][:, hB * 65:hB * 65 + 65],
                                         pB[:, kb * 128:(kb + 1) * 128],
                                         start=(kb == 0), stop=(kb == 1))
                    rzA = small.tile([1, 128], BF16, tag="rzA", name="rzA")
                    rzB = small.tile([1, 128], BF16, tag="rzB", name="rzB")
                    with nc.allow_low_precision("1/Z in bf16 is within tol"):
                        nc.vector.reciprocal(rzA[:], psA[64:65, :])
                        nc.vector.reciprocal(rzB[:], psB[64:65, :])
                    nc.tensor.matmul(bps[0:64, :], onesr[:], rzA[:],
                                     start=True, stop=True)
                    nc.tensor.matmul(bps[64:128, :], onesr[:], rzB[:],
                                     start=True, stop=True)
                    bsb = soft.tile([128, 128], BF16, tag="bsb", name="bsb")
                    nc.scalar.activation(bsb[:], bps,
                                         mybir.ActivationFunctionType.Copy)
                    nc.vector.tensor_mul(
                        a_sb[g][0:64, t * 128:(t + 1) * 128],
                        psA[0:64, :], bsb[0:64, :])
                    nc.vector.tensor_mul(
                        a_sb[g][64:128, t * 128:(t + 1) * 128],
                        psB[0:64, :], bsb[64:128, :])

            # ---- output projection (tail, reuses pp banks) ----------
            for gg in range(4):
                ps = pp.tile([128, 512], F32, tag="pp", name="pp")
                for g in range(4):
                    nc.tensor.matmul(ps[:, :SEQ_PER_CORE],
                                     wo_sb[:, g * 512 + gg * 128:
                                           g * 512 + gg * 128 + 128],
                                     a_sb[g][:],
                                     start=(g == 0), stop=(g == 3))
                nc.vector.tensor_scalar_add(o_sb[gg][:], ps[:, :SEQ_PER_CORE],
                                            bias_sb[:, 4 + gg:5 + gg])
                nc.sync.dma_start(outT[:, gg * 256:(gg + 1) * 256], o_sb[gg][:])

    nc.compile()
    return nc


def _band_mask(T):
    """Binary S^T masks [128 keys, 256 (2 kb blocks of 128)] for global
    query tile T (0..15)."""
    j = np.arange(128)[:, None]
    r = np.arange(128)[None, :]
    out = np.zeros((128, 256), np.float32)
    for kb in range(2):
        band = (j >= r) if kb == 0 else (j <= r)
        jg = T * 128 - W + kb * 128 + j
        valid = band & (jg >= 0) & (jg < S)
        out[:, kb * 128:(kb + 1) * 128] = valid
    return out


def _prep_inputs(x, Wq, bq, Wk, bk, Wv, bv, Wo, bo):
    bf = ml_dtypes.bfloat16
    f32 = np.float32

    def pack_w(Wm, scale=1.0):
        wT = np.asarray(Wm, f32).T * scale          # [512 in, 512 out]
        return np.ascontiguousarray(
            wT.reshape(4, 128, 512).transpose(1, 0, 2).reshape(128, 2048)
            .astype(bf))

    wk4 = pack_w(Wk)
    wq4 = pack_w(Wq, SCALE)
    wv4 = pack_w(Wv)
    wo4 = pack_w(Wo)
    bo_eff = np.asarray(bo, f32) + np.asarray(Wo, f32) @ np.asarray(bv, f32)
    bias = np.zeros((128, 12), f32)
    bias[:, 0:4] = np.asarray(bk, f32).reshape(4, 128).T
    bias[:, 4:8] = bo_eff.reshape(4, 128).T
    bias[:, 8:12] = (np.asarray(bq, f32) * SCALE).reshape(4, 128).T

    xf = np.asarray(x, f32)
    in_maps = []
    for c in range(N_CORES):
        klo = c * SEQ_PER_CORE - W
        lo, hi = max(0, klo), min(S, klo + CHUNK)
        xT_c = np.zeros((D, CHUNK), f32)
        xT_c[:, lo - klo:hi - klo] = xf[0, lo:hi, :].T
        x4 = np.ascontiguousarray(
            xT_c.reshape(4, 128, CHUNK).transpose(1, 0, 2)
            .reshape(128, 4 * CHUNK).astype(bf))
        m = np.concatenate([_band_mask(c * 2), _band_mask(c * 2 + 1)],
                           axis=1).astype(bf)
        in_maps.append({
            "x4": x4, "mask4": np.ascontiguousarray(m), "bias": bias,
            "wq4": wq4, "wk4": wk4, "wv4": wv4, "wo4": wo4,
        })
    return in_maps


def kernel(x, Wq, bq, Wk, bk, Wv, bv, Wo, bo):
    if "nc" not in _CACHE:
        _CACHE["nc"] = _build_program()
    nc = _CACHE["nc"]
    in_maps = _prep_inputs(x, Wq, bq, Wk, bk, Wv, bv, Wo, bo)
    res = bass_utils.run_bass_kernel_spmd(nc, in_maps,
                                          core_ids=list(range(N_CORES)))
    out = np.empty((1, S, D), np.float32)
    for c in range(N_CORES):
        arr = np.asarray(res.results[c]["outT"]).astype(np.float32)
        chunk = arr.reshape(128, 4, SEQ_PER_CORE).transpose(1, 0, 2) \
                   .reshape(D, SEQ_PER_CORE).T
        out[0, c * SEQ_PER_CORE:(c + 1) * SEQ_PER_CORE, :] = chunk
    return out


# revision 16
# speedup vs baseline: 1.1606x; 1.1517x over previous
"""Locally banded sparse attention (window=64) on 8 Trainium2 NeuronCores.

Sequence-parallel: each core owns 256 contiguous query positions and
receives a 384-row x chunk (its 256 rows + 64-row halo on each side,
zero-padded at the sequence edges) plus a full replica of the four
projection matrices.  No device collectives are needed.

All matmuls run in bf16 (fp32 PSUM accumulation).  Attention scores are
computed directly in transposed layout S^T[key, query] = kT.T @ qT, and
the P@V matmul uses P^T as the stationary operand so its output lands
query-major: av[q, d] with the softmax denominator Z[q] riding along as a
ones-column of V (col 64 of each head's 65-wide slot).  Normalization is
then a per-partition reciprocal + tensor_scalar multiply — no partition
broadcasts anywhere.  The q-major attention output is PE-transposed back
to d-major for the output projection.

Host-side folds: SCALE and bq into Wq/bq, bv into an effective bo
(out += bv @ Wo.T is query-independent).

Engine balance: PE matmuls; ACT exp + kT/o copies; DVE qT/vaug copies,
reciprocals, normalization, transpose copies; GPSIMD band-mask multiplies;
inputs DMA'd on two HWDGE queues (sync + scalar) in consumption order.
"""

import numpy as np
import ml_dtypes

import concourse.bass as bass
import concourse.tile as tile
from concourse import bacc, mybir
from concourse import bass_utils
from concourse.masks import make_identity

F32 = mybir.dt.float32
BF16 = mybir.dt.bfloat16
N_CORES = 8
S = 2048
D = 512
H = 8
DK = 64
W = 64
SCALE = 1.0 / np.sqrt(DK)
SEQ_PER_CORE = S // N_CORES          # 256
CHUNK = SEQ_PER_CORE + 2 * W         # 384 rows of k/v context per core

_CACHE = {}


def _build_program():
    nc = bacc.Bacc("TRN2", target_bir_lowering=False, debug=False,
                   num_devices=N_CORES)

    # packed DRAM inputs (bf16): weight chunk kk lives at cols [kk*512, +512)
    x4 = nc.dram_tensor("x4", [128, 4 * CHUNK], BF16, kind="ExternalInput").ap()
    wk4 = nc.dram_tensor("wk4", [128, 2048], BF16, kind="ExternalInput").ap()
    wq4 = nc.dram_tensor("wq4", [128, 2048], BF16, kind="ExternalInput").ap()
    wv4 = nc.dram_tensor("wv4", [128, 2048], BF16, kind="ExternalInput").ap()
    wo4 = nc.dram_tensor("wo4", [128, 2048], BF16, kind="ExternalInput").ap()
    # binary band masks, S^T layout: block (t, kb) at cols [(t*2+kb)*128)
    mask4 = nc.dram_tensor("mask4", [128, 512], BF16, kind="ExternalInput").ap()
    # col 0-3: bk[g]; col 4-7: bo_eff[g]; col 8-11: bq_scaled[g]
    bias = nc.dram_tensor("bias", [128, 12], F32, kind="ExternalInput").ap()
    outT = nc.dram_tensor("outT", [128, 4 * SEQ_PER_CORE], BF16,
                          kind="ExternalOutput").ap()

    with tile.TileContext(nc) as tc:
        with (
            tc.tile_pool(name="const", bufs=1) as cpool,
            tc.tile_pool(name="pp", bufs=2, space="PSUM") as pp,
            tc.tile_pool(name="s_ps", bufs=4, space="PSUM") as s_ps,
            tc.tile_pool(name="av_ps", bufs=2, space="PSUM") as av_ps,
            tc.tile_pool(name="soft", bufs=4) as soft,
            tc.tile_pool(name="small", bufs=4) as small,
        ):
            def persist(shape, tag, dtype=BF16):
                return cpool.tile(shape, dtype, tag=tag, name=tag)

            x_sb = persist([128, 4 * CHUNK], "x")
            wk_sb = persist([128, 2048], "wk")
            wq_sb = persist([128, 2048], "wq")
            wv_sb = persist([128, 2048], "wv")
            wo_sb = persist([128, 2048], "wo")
            mask_sb = persist([128, 512], "mask")
            bias_sb = persist([128, 12], "bias", F32)
            k_sb = [persist([128, CHUNK], f"k{g}") for g in range(4)]
            q_sb = [persist([128, SEQ_PER_CORE], f"q{g}") for g in range(4)]
            # v with a ones column per head: head h at cols [h*65, +64], Z at h*65+64
            vaug = [persist([128, 8 * 65], f"v{r}") for r in range(3)]
            aT_sb = [persist([128, D], f"aT{t}") for t in range(2)]
            a_sb = [persist([128, SEQ_PER_CORE], f"a{g}") for g in range(4)]
            o_sb = [persist([128, SEQ_PER_CORE], f"o{g}") for g in range(4)]
            ident = persist([128, 128], "ident")

            # input DMAs on two HWDGE queues, consumption order
            nc.sync.dma_start(x_sb[:], x4[:, :])
            nc.sync.dma_start(wq_sb[:], wq4[:, :])
            nc.sync.dma_start(wo_sb[:], wo4[:, :])
            nc.scalar.dma_start(bias_sb[:], bias[:, :])
            nc.scalar.dma_start(wk_sb[:], wk4[:, :])
            nc.scalar.dma_start(wv_sb[:], wv4[:, :])
            nc.scalar.dma_start(mask_sb[:], mask4[:, :])

            make_identity(nc, ident[:])

            def vaug_ap(r, col0, ncols):
                base = vaug[r][:]
                p_step = base.ap[0][0]
                return bass.AP(base.tensor, base.offset + col0,
                               [[p_step, 128], [65, 8], [1, ncols]])

            for r in range(3):
                nc.gpsimd.memset(vaug_ap(r, 64, 1), 1.0)

            # ---- projections ----------------------------------------
            # kT[g]: [128 dout, 384 keys], bf16, + bk   (copy on ACT)
            for g in range(4):
                ps = pp.tile([128, 512], F32, tag="pp", name="pp")
                for kk in range(4):
                    nc.tensor.matmul(ps[:, :CHUNK],
                                     wk_sb[:, kk * 512 + g * 128:
                                           kk * 512 + g * 128 + 128],
                                     x_sb[:, kk * CHUNK:(kk + 1) * CHUNK],
                                     start=(kk == 0), stop=(kk == 3))
                nc.scalar.activation(k_sb[g][:], ps[:, :CHUNK],
                                     mybir.ActivationFunctionType.Identity,
                                     bias=bias_sb[:, g:g + 1])
            # qT[g]: [128 dout, 256 queries] (SCALE, bq folded) (copy on DVE)
            for g in range(4):
                ps = pp.tile([128, 512], F32, tag="pp", name="pp")
                for kk in range(4):
                    nc.tensor.matmul(ps[:, :SEQ_PER_CORE],
                                     wq_sb[:, kk * 512 + g * 128:
                                           kk * 512 + g * 128 + 128],
                                     x_sb[:, kk * CHUNK + W:
                                          kk * CHUNK + W + SEQ_PER_CORE],
                                     start=(kk == 0), stop=(kk == 3))
                nc.vector.tensor_scalar_add(q_sb[g][:], ps[:, :SEQ_PER_CORE],
                                            bias_sb[:, 8 + g:9 + g])
            # v natural [keys, dout] -> vaug 65-wide head slots (copy on DVE)
            for r in range(3):
                ps = pp.tile([128, 512], F32, tag="pp", name="pp")
                for kk in range(4):
                    nc.tensor.matmul(ps[:],
                                     x_sb[:, kk * CHUNK + r * 128:
                                          kk * CHUNK + r * 128 + 128],
                                     wv_sb[:, kk * 512:(kk + 1) * 512],
                                     start=(kk == 0), stop=(kk == 3))
                nc.vector.tensor_copy(vaug_ap(r, 0, 64), ps[:])

            # ---- banded attention (S^T scores, q-major AV) ----------
            # software pipeline: S^T for step i runs on PE while step i-1
            # finishes softmax on ACT/GPSIMD, then its AV matmuls issue.
            steps = [(g, t) for g in range(4) for t in range(2)]
            pend = None   # (g, t, pA, pB, avz)

            def emit_av(st):
                g, t, pA, pB, avz = st
                hA, hB = 2 * g, 2 * g + 1
                for kb in range(2):
                    nc.tensor.matmul(avz[:, 0:65],
                                     pA[:, kb * 128:(kb + 1) * 128],
                                     vaug[t + kb][:, hA * 65:hA * 65 + 65],
                                     start=(kb == 0), stop=(kb == 1))
                for kb in range(2):
                    nc.tensor.matmul(avz[:, 65:130],
                                     pB[:, kb * 128:(kb + 1) * 128],
                                     vaug[t + kb][:, hB * 65:hB * 65 + 65],
                                     start=(kb == 0), stop=(kb == 1))
                rz2 = small.tile([128, 2], F32, tag="rz", name="rz")
                zbase = avz[:]
                zin = bass.AP(zbase.tensor, zbase.offset + 64,
                              [[zbase.ap[0][0], 128], [65, 2]])
                nc.vector.reciprocal(rz2[:], zin)
                nc.vector.tensor_scalar_mul(aT_sb[t][:, hA * 64:hA * 64 + 64],
                                            avz[:, 0:64], rz2[:, 0:1])
                nc.vector.tensor_scalar_mul(aT_sb[t][:, hB * 64:hB * 64 + 64],
                                            avz[:, 65:129], rz2[:, 1:2])

            for g, t in steps:
                sA = s_ps.tile([128, 256], F32, tag="s", name="s")
                sB = s_ps.tile([128, 256], F32, tag="s", name="s")
                for kb in range(2):
                    kc = (t + kb) * 128
                    nc.tensor.matmul(sA[:, kb * 128:(kb + 1) * 128],
                                     k_sb[g][0:64, kc:kc + 128],
                                     q_sb[g][0:64, t * 128:(t + 1) * 128],
                                     start=True, stop=True)
                    nc.tensor.matmul(sB[:, kb * 128:(kb + 1) * 128],
                                     k_sb[g][64:128, kc:kc + 128],
                                     q_sb[g][64:128, t * 128:(t + 1) * 128],
                                     start=True, stop=True)
                eA = soft.tile([128, 256], BF16, tag="eA", name="eA")
                eB = soft.tile([128, 256], BF16, tag="eB", name="eB")
                nc.scalar.activation(eA[:], sA[:],
                                     mybir.ActivationFunctionType.Exp)
                nc.scalar.activation(eB[:], sB[:],
                                     mybir.ActivationFunctionType.Exp)
                pA = soft.tile([128, 256], BF16, tag="pA", name="pA")
                pB = soft.tile([128, 256], BF16, tag="pB", name="pB")
                msk = mask_sb[:, t * 256:(t + 1) * 256]
                nc.gpsimd.tensor_mul(pA[:], eA[:], msk)
                nc.gpsimd.tensor_mul(pB[:], eB[:], msk)
                avz = av_ps.tile([128, 130], F32, tag="av", name="av")
                if pend is not None:
                    emit_av(pend)
                pend = (g, t, pA, pB, avz)
            emit_av(pend)

            # ---- transpose a^T back to d-major ----------------------
            for t in range(2):
                for g in range(4):
                    tp = av_ps.tile([128, 128], BF16, tag="av", name="tp")
                    nc.tensor.transpose(tp[:],
                                        aT_sb[t][:, g * 128:(g + 1) * 128],
                                        ident[:])
                    nc.vector.tensor_copy(a_sb[g][:, t * 128:(t + 1) * 128],
                                          tp[:])

            # ---- output projection (tail, reuses pp banks) ----------
            for gg in range(4):
                ps = pp.tile([128, 512], F32, tag="pp", name="pp")
                for g in range(4):
                    nc.tensor.matmul(ps[:, :SEQ_PER_CORE],
                                     wo_sb[:, g * 512 + gg * 128:
                                           g * 512 + gg * 128 + 128],
                                     a_sb[g][:],
                                     start=(g == 0), stop=(g == 3))
                nc.scalar.activation(o_sb[gg][:], ps[:, :SEQ_PER_CORE],
                                     mybir.ActivationFunctionType.Identity,
                                     bias=bias_sb[:, 4 + gg:5 + gg])
                nc.sync.dma_start(outT[:, gg * 256:(gg + 1) * 256], o_sb[gg][:])

    nc.compile()
    return nc


def _band_mask(T):
    """Binary S^T masks [128 keys, 256 (2 kb blocks of 128)] for global
    query tile T (0..15)."""
    j = np.arange(128)[:, None]
    r = np.arange(128)[None, :]
    out = np.zeros((128, 256), np.float32)
    for kb in range(2):
        band = (j >= r) if kb == 0 else (j <= r)
        jg = T * 128 - W + kb * 128 + j
        valid = band & (jg >= 0) & (jg < S)
        out[:, kb * 128:(kb + 1) * 128] = valid
    return out


def _prep_inputs(x, Wq, bq, Wk, bk, Wv, bv, Wo, bo):
    bf = ml_dtypes.bfloat16
    f32 = np.float32

    def pack_w(Wm, scale=1.0):
        wT = np.asarray(Wm, f32).T * scale          # [512 in, 512 out]
        return np.ascontiguousarray(
            wT.reshape(4, 128, 512).transpose(1, 0, 2).reshape(128, 2048)
            .astype(bf))

    wk4 = pack_w(Wk)
    wq4 = pack_w(Wq, SCALE)
    wv4 = pack_w(Wv)
    wo4 = pack_w(Wo)
    bo_eff = np.asarray(bo, f32) + np.asarray(Wo, f32) @ np.asarray(bv, f32)
    bias = np.zeros((128, 12), f32)
    bias[:, 0:4] = np.asarray(bk, f32).reshape(4, 128).T
    bias[:, 4:8] = bo_eff.reshape(4, 128).T
    bias[:, 8:12] = (np.asarray(bq, f32) * SCALE).reshape(4, 128).T

    xf = np.asarray(x, f32)
    in_maps = []
    for c in range(N_CORES):
        klo = c * SEQ_PER_CORE - W
        lo, hi = max(0, klo), min(S, klo + CHUNK)
        xT_c = np.zeros((D, CHUNK), f32)
        xT_c[:, lo - klo:hi - klo] = xf[0, lo:hi, :].T
        x4 = np.ascontiguousarray(
            xT_c.reshape(4, 128, CHUNK).transpose(1, 0, 2)
            .reshape(128, 4 * CHUNK).astype(bf))
        m = np.concatenate([_band_mask(c * 2), _band_mask(c * 2 + 1)],
                           axis=1).astype(bf)
        in_maps.append({
            "x4": x4, "mask4": np.ascontiguousarray(m), "bias": bias,
            "wq4": wq4, "wk4": wk4, "wv4": wv4, "wo4": wo4,
        })
    return in_maps


def kernel(x, Wq, bq, Wk, bk, Wv, bv, Wo, bo):
    if "nc" not in _CACHE:
        _CACHE["nc"] = _build_program()
    nc = _CACHE["nc"]
    in_maps = _prep_inputs(x, Wq, bq, Wk, bk, Wv, bv, Wo, bo)
    res = bass_utils.run_bass_kernel_spmd(nc, in_maps,
                                          core_ids=list(range(N_CORES)))
    out = np.empty((1, S, D), np.float32)
    for c in range(N_CORES):
        arr = np.asarray(res.results[c]["outT"]).astype(np.float32)
        chunk = arr.reshape(128, 4, SEQ_PER_CORE).transpose(1, 0, 2) \
                   .reshape(D, SEQ_PER_CORE).T
        out[0, c * SEQ_PER_CORE:(c + 1) * SEQ_PER_CORE, :] = chunk
    return out


# revision 22
# speedup vs baseline: 1.3320x; 1.1476x over previous
"""Locally banded sparse attention (window=64) on 8 Trainium2 NeuronCores.

Sequence-parallel: each core owns 256 contiguous query positions and
receives a 384-row x chunk (its 256 rows + 64-row halo on each side,
zero-padded at the sequence edges) plus a full replica of the four
projection matrices.  No device collectives are needed.

All matmuls run in bf16 (fp32 PSUM accumulation).  Attention scores are
computed directly in transposed layout S^T[key, query] = kT.T @ qT, and
the P@V matmul uses P^T as the stationary operand so its output lands
query-major: av[q, d] with the softmax denominator Z[q] riding along as a
ones-column of V (col 64 of each head's 65-wide slot).  Normalization is
then a per-partition reciprocal + tensor_scalar multiply — no partition
broadcasts anywhere.  The q-major attention output is PE-transposed back
to d-major for the output projection.

Host-side folds: SCALE and bq into Wq/bq, bv into an effective bo
(out += bv @ Wo.T is query-independent).

Engine balance: PE matmuls; ACT exp + kT/o copies; DVE qT/vaug copies,
reciprocals, normalization, transpose copies; GPSIMD band-mask multiplies;
inputs DMA'd on two HWDGE queues (sync + scalar) in consumption order.
"""

import numpy as np
import ml_dtypes

import concourse.bass as bass
import concourse.tile as tile
from concourse import bacc, mybir
from concourse import bass_utils

F32 = mybir.dt.float32
BF16 = mybir.dt.bfloat16
N_CORES = 8
S = 2048
D = 512
H = 8
DK = 64
W = 64
SCALE = 1.0 / np.sqrt(DK)
SEQ_PER_CORE = S // N_CORES          # 256
CHUNK = SEQ_PER_CORE + 2 * W         # 384 rows of k/v context per core

_CACHE = {}


def _build_program():
    nc = bacc.Bacc("TRN2", target_bir_lowering=False, debug=False,
                   num_devices=N_CORES)

    # packed DRAM inputs (bf16): weight chunk kk lives at cols [kk*512, +512)
    x4 = nc.dram_tensor("x4", [128, 4 * CHUNK], BF16, kind="ExternalInput").ap()
    wk4 = nc.dram_tensor("wk4", [128, 2048], BF16, kind="ExternalInput").ap()
    wq4 = nc.dram_tensor("wq4", [128, 2048], BF16, kind="ExternalInput").ap()
    wv4 = nc.dram_tensor("wv4", [128, 2048], BF16, kind="ExternalInput").ap()
    wo4 = nc.dram_tensor("wo4", [128, 2048], BF16, kind="ExternalInput").ap()
    # binary band masks, S^T layout: block (t, kb) at cols [(t*2+kb)*128)
    mask4 = nc.dram_tensor("mask4", [128, 512], BF16, kind="ExternalInput").ap()
    # col 0-3: bk[g]; col 4-7: bo_eff[g]; col 8-11: bq_scaled[g]
    bias = nc.dram_tensor("bias", [128, 12], F32, kind="ExternalInput").ap()
    identw = nc.dram_tensor("identw", [128, 128], BF16, kind="ExternalInput").ap()
    outT = nc.dram_tensor("outT", [128, 4 * SEQ_PER_CORE], BF16,
                          kind="ExternalOutput").ap()

    with tile.TileContext(nc) as tc:
        with (
            tc.tile_pool(name="const", bufs=1) as cpool,
            tc.tile_pool(name="pp", bufs=2, space="PSUM") as pp,
            tc.tile_pool(name="s_ps", bufs=4, space="PSUM") as s_ps,
            tc.tile_pool(name="av_ps", bufs=2, space="PSUM") as av_ps,
            tc.tile_pool(name="soft", bufs=4) as soft,
            tc.tile_pool(name="small", bufs=4) as small,
        ):
            def persist(shape, tag, dtype=BF16):
                return cpool.tile(shape, dtype, tag=tag, name=tag)

            x_sb = persist([128, 4 * CHUNK], "x")
            wk_sb = persist([128, 2048], "wk")
            wq_sb = persist([128, 2048], "wq")
            wv_sb = persist([128, 2048], "wv")
            wo_sb = persist([128, 2048], "wo")
            mask_sb = persist([128, 512], "mask")
            bias_sb = persist([128, 12], "bias", F32)
            k_sb = [persist([128, CHUNK], f"k{g}") for g in range(4)]
            q_sb = [persist([128, SEQ_PER_CORE], f"q{g}") for g in range(4)]
            # v with a ones column per head: head h at cols [h*65, +64], Z at h*65+64
            vaug = [persist([128, 8 * 65], f"v{r}") for r in range(3)]
            aT_sb = [persist([128, D], f"aT{t}") for t in range(2)]
            a_sb = [persist([128, SEQ_PER_CORE], f"a{g}") for g in range(4)]
            o_sb = [persist([128, SEQ_PER_CORE], f"o{g}") for g in range(4)]
            ident = persist([128, 128], "ident")
            scratch = persist([128, 256], "scratch")

            # input DMAs on two HWDGE queues: x and wk race first in
            # parallel (SDMA round-robins across queues), rest behind them
            nc.sync.dma_start(x_sb[:], x4[:, :])
            nc.sync.dma_start(wq_sb[:], wq4[:, :])
            nc.sync.dma_start(ident[:], identw[:, :])
            nc.sync.dma_start(wo_sb[:], wo4[:, :])
            nc.scalar.dma_start(wk_sb[:], wk4[:, :])
            nc.scalar.dma_start(bias_sb[:], bias[:, :])
            nc.scalar.dma_start(wv_sb[:], wv4[:, :])
            nc.scalar.dma_start(mask_sb[:], mask4[:, :])

            def vaug_ap(r, col0, ncols):
                base = vaug[r][:]
                p_step = base.ap[0][0]
                return bass.AP(base.tensor, base.offset + col0,
                               [[p_step, 128], [65, 8], [1, ncols]])

            for r in range(3):
                nc.vector.memset(vaug_ap(r, 64, 1), 1.0)

            # HAM warm-up: keep the PE streaming dummy matmuls while the
            # weight DMAs land so real matmuls run at 2.4 GHz, not 1.2
            nc.vector.memset(scratch[:], 0.0)
            for w in range(44):
                wps = s_ps.tile([128, 256], F32, tag="s", name="warm")
                nc.tensor.matmul(wps[:], scratch[:, 0:128], scratch[:],
                                 start=True, stop=True)

            # ---- projections ----------------------------------------
            # kT[g]: [128 dout, 384 keys], bf16, + bk   (copy on ACT)
            for g in range(4):
                ps = pp.tile([128, 512], F32, tag="pp", name="pp")
                for kk in range(4):
                    nc.tensor.matmul(ps[:, :CHUNK],
                                     wk_sb[:, kk * 512 + g * 128:
                                           kk * 512 + g * 128 + 128],
                                     x_sb[:, kk * CHUNK:(kk + 1) * CHUNK],
                                     start=(kk == 0), stop=(kk == 3))
                nc.scalar.activation(k_sb[g][:], ps[:, :CHUNK],
                                     mybir.ActivationFunctionType.Identity,
                                     bias=bias_sb[:, g:g + 1])
            # qT[g]: [128 dout, 256 queries] (SCALE, bq folded) (copy on DVE)
            for g in range(4):
                ps = pp.tile([128, 512], F32, tag="pp", name="pp")
                for kk in range(4):
                    nc.tensor.matmul(ps[:, :SEQ_PER_CORE],
                                     wq_sb[:, kk * 512 + g * 128:
                                           kk * 512 + g * 128 + 128],
                                     x_sb[:, kk * CHUNK + W:
                                          kk * CHUNK + W + SEQ_PER_CORE],
                                     start=(kk == 0), stop=(kk == 3))
                nc.vector.tensor_scalar_add(q_sb[g][:], ps[:, :SEQ_PER_CORE],
                                            bias_sb[:, 8 + g:9 + g])
            # v natural [keys, dout] -> vaug 65-wide head slots (copy on DVE)
            for r in range(3):
                ps = pp.tile([128, 512], F32, tag="pp", name="pp")
                for kk in range(4):
                    nc.tensor.matmul(ps[:],
                                     x_sb[:, kk * CHUNK + r * 128:
                                          kk * CHUNK + r * 128 + 128],
                                     wv_sb[:, kk * 512:(kk + 1) * 512],
                                     start=(kk == 0), stop=(kk == 3))
                nc.vector.tensor_copy(vaug_ap(r, 0, 64), ps[:])

            # ---- banded attention (S^T scores, q-major AV) ----------
            # software pipeline: S^T for step i runs on PE while step i-1
            # finishes softmax on ACT/GPSIMD, then its AV matmuls issue.
            steps = [(g, t) for g in range(4) for t in range(2)]
            pend = None   # (g, t, pA, pB, avz)

            def emit_av(st):
                g, t, pA, pB, avz = st
                hA, hB = 2 * g, 2 * g + 1
                for kb in range(2):
                    nc.tensor.matmul(avz[:, 0:65],
                                     pA[:, kb * 128:(kb + 1) * 128],
                                     vaug[t + kb][:, hA * 65:hA * 65 + 65],
                                     start=(kb == 0), stop=(kb == 1))
                for kb in range(2):
                    nc.tensor.matmul(avz[:, 65:130],
                                     pB[:, kb * 128:(kb + 1) * 128],
                                     vaug[t + kb][:, hB * 65:hB * 65 + 65],
                                     start=(kb == 0), stop=(kb == 1))
                rz2 = small.tile([128, 2], F32, tag="rz", name="rz")
                zbase = avz[:]
                zin = bass.AP(zbase.tensor, zbase.offset + 64,
                              [[zbase.ap[0][0], 128], [65, 2]])
                nc.vector.reciprocal(rz2[:], zin)
                nc.vector.tensor_scalar_mul(aT_sb[t][:, hA * 64:hA * 64 + 64],
                                            avz[:, 0:64], rz2[:, 0:1])
                nc.vector.tensor_scalar_mul(aT_sb[t][:, hB * 64:hB * 64 + 64],
                                            avz[:, 65:129], rz2[:, 1:2])

            for g, t in steps:
                sA = s_ps.tile([128, 256], F32, tag="s", name="s")
                sB = s_ps.tile([128, 256], F32, tag="s", name="s")
                for kb in range(2):
                    kc = (t + kb) * 128
                    nc.tensor.matmul(sA[:, kb * 128:(kb + 1) * 128],
                                     k_sb[g][0:64, kc:kc + 128],
                                     q_sb[g][0:64, t * 128:(t + 1) * 128],
                                     start=True, stop=True)
                    nc.tensor.matmul(sB[:, kb * 128:(kb + 1) * 128],
                                     k_sb[g][64:128, kc:kc + 128],
                                     q_sb[g][64:128, t * 128:(t + 1) * 128],
                                     start=True, stop=True)
                eA = soft.tile([128, 256], BF16, tag="eA", name="eA")
                eB = soft.tile([128, 256], BF16, tag="eB", name="eB")
                nc.scalar.activation(eA[:], sA[:],
                                     mybir.ActivationFunctionType.Exp)
                nc.scalar.activation(eB[:], sB[:],
                                     mybir.ActivationFunctionType.Exp)
                pA = soft.tile([128, 256], BF16, tag="pA", name="pA")
                pB = soft.tile([128, 256], BF16, tag="pB", name="pB")
                msk = mask_sb[:, t * 256:(t + 1) * 256]
                nc.gpsimd.tensor_mul(pA[:], eA[:], msk)
                nc.vector.tensor_mul(pB[:], eB[:], msk)
                avz = av_ps.tile([128, 130], F32, tag="av", name="av")
                if pend is not None:
                    emit_av(pend)
                pend = (g, t, pA, pB, avz)
            emit_av(pend)

            # ---- transpose a^T back to d-major ----------------------
            for t in range(2):
                for g in range(4):
                    tp = av_ps.tile([128, 128], BF16, tag="av", name="tp")
                    nc.tensor.transpose(tp[:],
                                        aT_sb[t][:, g * 128:(g + 1) * 128],
                                        ident[:])
                    nc.vector.tensor_copy(a_sb[g][:, t * 128:(t + 1) * 128],
                                          tp[:])

            # ---- output projection (tail, reuses pp banks) ----------
            for gg in range(4):
                ps = pp.tile([128, 512], F32, tag="pp", name="pp")
                for g in range(4):
                    nc.tensor.matmul(ps[:, :SEQ_PER_CORE],
                                     wo_sb[:, g * 512 + gg * 128:
                                           g * 512 + gg * 128 + 128],
                                     a_sb[g][:],
                                     start=(g == 0), stop=(g == 3))
                nc.scalar.activation(o_sb[gg][:], ps[:, :SEQ_PER_CORE],
                                     mybir.ActivationFunctionType.Identity,
                                     bias=bias_sb[:, 4 + gg:5 + gg])
                nc.sync.dma_start(outT[:, gg * 256:(gg + 1) * 256], o_sb[gg][:])

    nc.compile()
    return nc


def _band_mask(T):
    """Binary S^T masks [128 keys, 256 (2 kb blocks of 128)] for global
    query tile T (0..15)."""
    j = np.arange(128)[:, None]
    r = np.arange(128)[None, :]
    out = np.zeros((128, 256), np.float32)
    for kb in range(2):
        band = (j >= r) if kb == 0 else (j <= r)
        jg = T * 128 - W + kb * 128 + j
        valid = band & (jg >= 0) & (jg < S)
        out[:, kb * 128:(kb + 1) * 128] = valid
    return out


def _prep_inputs(x, Wq, bq, Wk, bk, Wv, bv, Wo, bo):
    bf = ml_dtypes.bfloat16
    f32 = np.float32

    def pack_w(Wm, scale=1.0):
        wT = np.asarray(Wm, f32).T * scale          # [512 in, 512 out]
        return np.ascontiguousarray(
            wT.reshape(4, 128, 512).transpose(1, 0, 2).reshape(128, 2048)
            .astype(bf))

    wk4 = pack_w(Wk)
    wq4 = pack_w(Wq, SCALE)
    wv4 = pack_w(Wv)
    wo4 = pack_w(Wo)
    bo_eff = np.asarray(bo, f32) + np.asarray(Wo, f32) @ np.asarray(bv, f32)
    bias = np.zeros((128, 12), f32)
    bias[:, 0:4] = np.asarray(bk, f32).reshape(4, 128).T
    bias[:, 4:8] = bo_eff.reshape(4, 128).T
    bias[:, 8:12] = (np.asarray(bq, f32) * SCALE).reshape(4, 128).T

    identw = np.eye(128, dtype=f32).astype(bf)
    xf = np.asarray(x, f32)
    in_maps = []
    for c in range(N_CORES):
        klo = c * SEQ_PER_CORE - W
        lo, hi = max(0, klo), min(S, klo + CHUNK)
        xT_c = np.zeros((D, CHUNK), f32)
        xT_c[:, lo - klo:hi - klo] = xf[0, lo:hi, :].T
        x4 = np.ascontiguousarray(
            xT_c.reshape(4, 128, CHUNK).transpose(1, 0, 2)
            .reshape(128, 4 * CHUNK).astype(bf))
        m = np.concatenate([_band_mask(c * 2), _band_mask(c * 2 + 1)],
                           axis=1).astype(bf)
        in_maps.append({
            "x4": x4, "mask4": np.ascontiguousarray(m), "bias": bias,
            "wq4": wq4, "wk4": wk4, "wv4": wv4, "wo4": wo4,
            "identw": identw,
        })
    return in_maps


def kernel(x, Wq, bq, Wk, bk, Wv, bv, Wo, bo):
    if "nc" not in _CACHE:
        _CACHE["nc"] = _build_program()
    nc = _CACHE["nc"]
    in_maps = _prep_inputs(x, Wq, bq, Wk, bk, Wv, bv, Wo, bo)
    res = bass_utils.run_bass_kernel_spmd(nc, in_maps,
                                          core_ids=list(range(N_CORES)))
    out = np.empty((1, S, D), np.float32)
    for c in range(N_CORES):
        arr = np.asarray(res.results[c]["outT"]).astype(np.float32)
        chunk = arr.reshape(128, 4, SEQ_PER_CORE).transpose(1, 0, 2) \
                   .reshape(D, SEQ_PER_CORE).T
        out[0, c * SEQ_PER_CORE:(c + 1) * SEQ_PER_CORE, :] = chunk
    return out
